# revision 10
# baseline (speedup 1.0000x reference)
"""Spectral pooling (FFT2 -> crop low freqs -> IFFT2) as dense DFT matmuls on TRN2.

Input  x: (32, 256, 64, 64) fp32  -- channels 0:128 real part, 128:256 imag part
Output y: (32, 256, 32, 32) fp32

Math: per complex image X (64x64), Y = A @ X @ A.T with
  A = (1/sqrt(64*64*32*32)**0.5) * IDFT32 @ Crop @ DFT64   (32x64 complex)
Sharding: batch dim across 8 cores (4 batches/core), no communication.

On-chip scheme (all matmuls data-stationary, bf16, K=128):
  stage 1: lhsT = [Xr_c; Xr_{c+1}] (two channels stacked on partitions),
           rhs  = blockdiag([Ar.T|Ai.T]) / blockdiag([-Ai.T|Ar.T]) -> P.T tiles
  stage 2: lhsT = P.T column slices (two channels stacked), same rhs -> Y
  tile_position col-groups pack 2 images per PE pass; PSUM -> SBUF copies on
  DVE/ACT; fp32->bf16 cast happens inside the SWDGE load DMA.
"""

import math

import numpy as np

from concourse import bass, mybir
from concourse.bass_utils import run_bass_kernel_spmd
from concourse.tile import TileContext

N_CORES = 8
B_FULL, C2, H, W = 32, 256, 64, 64
HP, WP = 32, 32
BPC = B_FULL // N_CORES  # batches per core

F32 = mybir.dt.float32
BF16 = mybir.dt.bfloat16


def _split_multi_waits(nc):
    """This walrus build rejects instructions carrying more than one semaphore
    wait. Hoist extra waits onto same-engine NOPs inserted just before the
    instruction (engine queues execute in order, so blocking is equivalent)."""
    n_split = 0
    for f in nc.m.functions:
        for bb in f.blocks:
            insts = bb.instructions
            out = []
            for inst in insts:
                si = inst.sync_info
                waits = list(si.on_wait) if si and si.on_wait else []
                if len(waits) > 1:
                    si.on_wait = waits[-1:]
                    for w in waits[:-1]:
                        nop = mybir.InstNoOp(
                            name=nc.get_next_instruction_name(),
                            ins=[],
                            outs=[],
                            engine=inst.engine,
                            sync_info=mybir.SyncInfo(on_wait=[w], on_update=[]),
                        )
                        out.append(nop)
                        n_split += 1
                out.append(inst)
            if len(out) != len(insts):
                insts[:] = out
    return n_split


def _dft_constants():
    """Dr, Di: [128,128] fp32 block-diag moving operands for both stages."""
    topf = int(math.ceil(H * 0.5 / 2))  # 16
    midf = H // 2 + topf  # 48
    F = np.exp(-2j * np.pi * np.outer(np.arange(H), np.arange(H)) / H)
    G = np.exp(2j * np.pi * np.outer(np.arange(HP), np.arange(HP)) / HP)
    keep = list(range(topf)) + list(range(midf, H))
    S = np.zeros((HP, H))
    S[np.arange(HP), keep] = 1
    A = (G @ S @ F) / np.sqrt(H * W * HP * WP) ** 0.5
    Ar = A.real.astype(np.float32)
    Ai = A.imag.astype(np.float32)
    C1r = np.concatenate([Ar.T, Ai.T], axis=1)  # [64, 64]
    C1i = np.concatenate([-Ai.T, Ar.T], axis=1)  # [64, 64]
    Dr = np.zeros((128, 128), np.float32)
    Di = np.zeros((128, 128), np.float32)
    Dr[:64, :64] = C1r
    Dr[64:, 64:] = C1r
    Di[:64, :64] = C1i
    Di[64:, 64:] = C1i
    return Dr, Di


def build_program():
    nc = bass.Bass("TRN2", target_bir_lowering=False, debug=False)
    x = nc.dram_tensor("x", [BPC, C2, H, W], F32, kind="ExternalInput").ap()
    dr = nc.dram_tensor("dr", [128, 128], F32, kind="ExternalInput").ap()
    di = nc.dram_tensor("di", [128, 128], F32, kind="ExternalInput").ap()
    y = nc.dram_tensor("y", [BPC, C2, HP, WP], F32, kind="ExternalOutput").ap()

    with TileContext(nc) as tc:
        with (
            tc.tile_pool(name="consts", bufs=1) as cpool,
            tc.tile_pool(name="inp", bufs=2) as ipool,
            tc.tile_pool(name="sb1", bufs=4) as s1pool,
            tc.tile_pool(name="sbout", bufs=2) as opool,
            tc.tile_pool(name="ps1", bufs=4, space="PSUM") as p1pool,
            tc.tile_pool(name="ps2", bufs=2, space="PSUM") as p2pool,
        ):
            drf = cpool.tile([128, 128], F32, tag="drf")
            nc.sync.dma_start(out=drf, in_=dr)
            dif = cpool.tile([128, 128], F32, tag="dif")
            nc.sync.dma_start(out=dif, in_=di)
            drb = cpool.tile([128, 128], BF16, tag="drb")
            nc.vector.tensor_copy(out=drb, in_=drf)
            dib = cpool.tile([128, 128], BF16, tag="dib")
            nc.vector.tensor_copy(out=dib, in_=dif)

            for b in range(BPC):
                # partitions = (channel parity, h); free = (pair, w)
                in_r = ipool.tile([128, (C2 // 4) * W], BF16, tag="in_r")
                nc.gpsimd.dma_start(
                    out=in_r,
                    in_=x[b, 0 : C2 // 2]
                    .rearrange("(pair two) h w -> pair two h w", two=2)
                    .transpose([1, 2, 0, 3]),
                )
                in_i = ipool.tile([128, (C2 // 4) * W], BF16, tag="in_i")
                nc.gpsimd.dma_start(
                    out=in_i,
                    in_=x[b, C2 // 2 : C2]
                    .rearrange("(pair two) h w -> pair two h w", two=2)
                    .transpose([1, 2, 0, 3]),
                )
                sb_out = opool.tile([128, 2048], F32, tag="sb_out")
                psum2 = None
                for q in range(C2 // 8):
                    # quad (o, qp): complex channels {8o+2qp, +1, +4, +5}
                    o, qp = q // 2, q % 2
                    pA = 4 * o + qp
                    pB = pA + 2
                    psum1 = p1pool.tile([128, 128], F32, tag="ps1")
                    nc.tensor.matmul(
                        out=psum1[0:64, :],
                        lhsT=in_r[:, 64 * pA : 64 * pA + 64],
                        rhs=drb,
                        start=True,
                        stop=False,
                        tile_position=(0, 0),
                    )
                    nc.tensor.matmul(
                        out=psum1[0:64, :],
                        lhsT=in_i[:, 64 * pA : 64 * pA + 64],
                        rhs=dib,
                        start=False,
                        stop=True,
                        tile_position=(0, 0),
                    )
                    nc.tensor.matmul(
                        out=psum1[64:128, :],
                        lhsT=in_r[:, 64 * pB : 64 * pB + 64],
                        rhs=drb,
                        start=True,
                        stop=False,
                        tile_position=(0, 64),
                    )
                    nc.tensor.matmul(
                        out=psum1[64:128, :],
                        lhsT=in_i[:, 64 * pB : 64 * pB + 64],
                        rhs=dib,
                        start=False,
                        stop=True,
                        tile_position=(0, 64),
                    )
                    sb1 = s1pool.tile([128, 128], BF16, tag="sb1")
                    nc.vector.tensor_copy(out=sb1, in_=psum1)

                    if q % 2 == 0:
                        psum2 = p2pool.tile([128, 128], F32, tag="ps2")
                    cb = 64 * (q % 2)
                    nc.tensor.matmul(
                        out=psum2[cb : cb + 32, :],
                        lhsT=sb1[:, 0:32],
                        rhs=drb,
                        start=True,
                        stop=False,
                        tile_position=(0, cb),
                    )
                    nc.tensor.matmul(
                        out=psum2[cb : cb + 32, :],
                        lhsT=sb1[:, 32:64],
                        rhs=dib,
                        start=False,
                        stop=True,
                        tile_position=(0, cb),
                    )
                    nc.tensor.matmul(
                        out=psum2[cb + 32 : cb + 64, :],
                        lhsT=sb1[:, 64:96],
                        rhs=drb,
                        start=True,
                        stop=False,
                        tile_position=(0, cb + 32),
                    )
                    nc.tensor.matmul(
                        out=psum2[cb + 32 : cb + 64, :],
                        lhsT=sb1[:, 96:128],
                        rhs=dib,
                        start=False,
                        stop=True,
                        tile_position=(0, cb + 32),
                    )
                    if q % 2 == 1:
                        nc.scalar.copy(
                            out=sb_out[:, 128 * o : 128 * (o + 1)], in_=psum2
                        )
                # channel = 128*ri + 8*o + 4*t + 2*qp + s ; partitions (qp s h)
                sbv = sb_out.rearrange(
                    "p (o t ri w) -> p (o t) ri w", o=16, t=2, ri=2, w=WP
                )
                for ri in range(2):
                    nc.sync.dma_start(
                        out=y[b, 128 * ri : 128 * (ri + 1)].rearrange(
                            "(o t qp s) h w -> (qp s h) (o t) w",
                            o=16,
                            t=2,
                            qp=2,
                            s=2,
                        ),
                        in_=sbv[:, :, ri, :],
                    )
    _split_multi_waits(nc)
    return nc


_CACHED = {}


def _get_program():
    if "nc" not in _CACHED:
        _CACHED["nc"] = build_program()
        _CACHED["consts"] = _dft_constants()
    return _CACHED["nc"], _CACHED["consts"]


def kernel(x: np.ndarray) -> np.ndarray:
    assert x.shape == (B_FULL, C2, H, W) and x.dtype == np.float32
    nc, (Dr, Di) = _get_program()
    x = np.ascontiguousarray(x)
    in_maps = [
        {"x": x[BPC * k : BPC * (k + 1)], "dr": Dr, "di": Di}
        for k in range(N_CORES)
    ]
    res = run_bass_kernel_spmd(nc, in_maps, list(range(N_CORES)))
    out = np.concatenate(
        [res.results[k]["y"] for k in range(N_CORES)], axis=0
    )
    return out.astype(np.float32, copy=False)


if __name__ == "__main__":
    rng = np.random.default_rng(0)
    x = rng.standard_normal((B_FULL, C2, H, W)).astype(np.float32)
    y = kernel(x)
    print("kernel output", y.shape, y.dtype)


# revision 12
# speedup vs baseline: 20587.8197x; 20587.8197x over previous
"""Spectral pooling (FFT2 -> crop low freqs -> IFFT2) as dense DFT matmuls on TRN2.

Input  x: (32, 256, 64, 64) fp32  -- channels 0:128 real part, 128:256 imag part
Output y: (32, 256, 32, 32) fp32

Math: per complex image X (64x64), Y = A @ X @ A.T with
  A = (1/sqrt(64*64*32*32)**0.5) * IDFT32 @ Crop @ DFT64   (32x64 complex)
Sharding: batch dim across 8 cores (4 batches/core), no communication.

On-chip scheme (all matmuls data-stationary, bf16, K=128):
  stage 1: lhsT = [Xr_c; Xr_{c+1}] (two channels stacked on partitions),
           rhs  = blockdiag([Ar.T|Ai.T]) / blockdiag([-Ai.T|Ar.T]) -> P.T tiles
  stage 2: lhsT = P.T column slices (two channels stacked), same rhs -> Y
  tile_position col-groups pack 2 images per PE pass; PSUM -> SBUF copies on
  DVE/ACT; fp32->bf16 cast happens inside the SWDGE load DMA.
"""

import math

import numpy as np

from concourse import bass, mybir
from concourse.bass_utils import run_bass_kernel_spmd
from concourse.tile import TileContext

N_CORES = 8
B_FULL, C2, H, W = 32, 256, 64, 64
HP, WP = 32, 32
BPC = B_FULL // N_CORES  # batches per core

F32 = mybir.dt.float32
BF16 = mybir.dt.bfloat16


def _split_multi_waits(nc):
    """This walrus build rejects instructions carrying more than one semaphore
    wait. Hoist extra waits onto same-engine NOPs inserted just before the
    instruction (engine queues execute in order, so blocking is equivalent)."""
    n_split = 0
    for f in nc.m.functions:
        for bb in f.blocks:
            insts = bb.instructions
            out = []
            for inst in insts:
                si = inst.sync_info
                waits = list(si.on_wait) if si and si.on_wait else []
                if len(waits) > 1:
                    si.on_wait = waits[-1:]
                    for w in waits[:-1]:
                        nop = mybir.InstNoOp(
                            name=nc.get_next_instruction_name(),
                            ins=[],
                            outs=[],
                            engine=inst.engine,
                            sync_info=mybir.SyncInfo(on_wait=[w], on_update=[]),
                        )
                        out.append(nop)
                        n_split += 1
                out.append(inst)
            if len(out) != len(insts):
                insts[:] = out
    return n_split


def _dft_constants():
    """Dr, Di: [128,128] fp32 block-diag moving operands for both stages."""
    topf = int(math.ceil(H * 0.5 / 2))  # 16
    midf = H // 2 + topf  # 48
    F = np.exp(-2j * np.pi * np.outer(np.arange(H), np.arange(H)) / H)
    G = np.exp(2j * np.pi * np.outer(np.arange(HP), np.arange(HP)) / HP)
    keep = list(range(topf)) + list(range(midf, H))
    S = np.zeros((HP, H))
    S[np.arange(HP), keep] = 1
    A = (G @ S @ F) / np.sqrt(H * W * HP * WP) ** 0.5
    Ar = A.real.astype(np.float32)
    Ai = A.imag.astype(np.float32)
    C1r = np.concatenate([Ar.T, Ai.T], axis=1)  # [64, 64]
    C1i = np.concatenate([-Ai.T, Ar.T], axis=1)  # [64, 64]
    Dr = np.zeros((128, 128), np.float32)
    Di = np.zeros((128, 128), np.float32)
    Dr[:64, :64] = C1r
    Dr[64:, 64:] = C1r
    Di[:64, :64] = C1i
    Di[64:, 64:] = C1i
    return Dr, Di


def build_program(reps: int = 1):
    """reps > 1 repeats the whole pipeline in-NEFF (same data) so the
    marginal cost per rep can be measured without dispatch overhead."""
    nc = bass.Bass("TRN2", target_bir_lowering=False, debug=False)
    x = nc.dram_tensor("x", [BPC, C2, H, W], F32, kind="ExternalInput").ap()
    dr = nc.dram_tensor("dr", [128, 128], F32, kind="ExternalInput").ap()
    di = nc.dram_tensor("di", [128, 128], F32, kind="ExternalInput").ap()
    y = nc.dram_tensor("y", [BPC, C2, HP, WP], F32, kind="ExternalOutput").ap()

    with TileContext(nc) as tc:
        with (
            tc.tile_pool(name="consts", bufs=1) as cpool,
            tc.tile_pool(name="inp", bufs=2) as ipool,
            tc.tile_pool(name="sb1", bufs=4) as s1pool,
            tc.tile_pool(name="sbout", bufs=2) as opool,
            tc.tile_pool(name="ps1", bufs=4, space="PSUM") as p1pool,
            tc.tile_pool(name="ps2", bufs=2, space="PSUM") as p2pool,
        ):
            drf = cpool.tile([128, 128], F32, tag="drf")
            nc.sync.dma_start(out=drf, in_=dr)
            dif = cpool.tile([128, 128], F32, tag="dif")
            nc.sync.dma_start(out=dif, in_=di)
            drb = cpool.tile([128, 128], BF16, tag="drb")
            nc.vector.tensor_copy(out=drb, in_=drf)
            dib = cpool.tile([128, 128], BF16, tag="dib")
            nc.vector.tensor_copy(out=dib, in_=dif)

            for b in [b for _ in range(reps) for b in range(BPC)]:
                # partitions = (channel parity, h); free = (pair, w)
                in_r = ipool.tile([128, (C2 // 4) * W], BF16, tag="in_r")
                nc.gpsimd.dma_start(
                    out=in_r,
                    in_=x[b, 0 : C2 // 2]
                    .rearrange("(pair two) h w -> pair two h w", two=2)
                    .transpose([1, 2, 0, 3]),
                )
                in_i = ipool.tile([128, (C2 // 4) * W], BF16, tag="in_i")
                nc.gpsimd.dma_start(
                    out=in_i,
                    in_=x[b, C2 // 2 : C2]
                    .rearrange("(pair two) h w -> pair two h w", two=2)
                    .transpose([1, 2, 0, 3]),
                )
                sb_out = opool.tile([128, 2048], F32, tag="sb_out")
                psum2 = None
                for q in range(C2 // 8):
                    # quad (o, qp): complex channels {8o+2qp, +1, +4, +5}
                    o, qp = q // 2, q % 2
                    pA = 4 * o + qp
                    pB = pA + 2
                    psum1 = p1pool.tile([128, 128], F32, tag="ps1")
                    nc.tensor.matmul(
                        out=psum1[0:64, :],
                        lhsT=in_r[:, 64 * pA : 64 * pA + 64],
                        rhs=drb,
                        start=True,
                        stop=False,
                        tile_position=(0, 0),
                    )
                    nc.tensor.matmul(
                        out=psum1[0:64, :],
                        lhsT=in_i[:, 64 * pA : 64 * pA + 64],
                        rhs=dib,
                        start=False,
                        stop=True,
                        tile_position=(0, 0),
                    )
                    nc.tensor.matmul(
                        out=psum1[64:128, :],
                        lhsT=in_r[:, 64 * pB : 64 * pB + 64],
                        rhs=drb,
                        start=True,
                        stop=False,
                        tile_position=(0, 64),
                    )
                    nc.tensor.matmul(
                        out=psum1[64:128, :],
                        lhsT=in_i[:, 64 * pB : 64 * pB + 64],
                        rhs=dib,
                        start=False,
                        stop=True,
                        tile_position=(0, 64),
                    )
                    sb1 = s1pool.tile([128, 128], BF16, tag="sb1")
                    nc.vector.tensor_copy(out=sb1, in_=psum1)

                    if q % 2 == 0:
                        psum2 = p2pool.tile([128, 128], F32, tag="ps2")
                    cb = 64 * (q % 2)
                    nc.tensor.matmul(
                        out=psum2[cb : cb + 32, :],
                        lhsT=sb1[:, 0:32],
                        rhs=drb,
                        start=True,
                        stop=False,
                        tile_position=(0, cb),
                    )
                    nc.tensor.matmul(
                        out=psum2[cb : cb + 32, :],
                        lhsT=sb1[:, 32:64],
                        rhs=dib,
                        start=False,
                        stop=True,
                        tile_position=(0, cb),
                    )
                    nc.tensor.matmul(
                        out=psum2[cb + 32 : cb + 64, :],
                        lhsT=sb1[:, 64:96],
                        rhs=drb,
                        start=True,
                        stop=False,
                        tile_position=(0, cb + 32),
                    )
                    nc.tensor.matmul(
                        out=psum2[cb + 32 : cb + 64, :],
                        lhsT=sb1[:, 96:128],
                        rhs=dib,
                        start=False,
                        stop=True,
                        tile_position=(0, cb + 32),
                    )
                    if q % 2 == 1:
                        nc.scalar.copy(
                            out=sb_out[:, 128 * o : 128 * (o + 1)], in_=psum2
                        )
                # channel = 128*ri + 8*o + 4*t + 2*qp + s ; partitions (qp s h)
                sbv = sb_out.rearrange(
                    "p (o t ri w) -> p (o t) ri w", o=16, t=2, ri=2, w=WP
                )
                for ri in range(2):
                    nc.sync.dma_start(
                        out=y[b, 128 * ri : 128 * (ri + 1)].rearrange(
                            "(o t qp s) h w -> (qp s h) (o t) w",
                            o=16,
                            t=2,
                            qp=2,
                            s=2,
                        ),
                        in_=sbv[:, :, ri, :],
                    )
    _split_multi_waits(nc)
    return nc


_CACHED = {}


def _get_program():
    if "nc" not in _CACHED:
        _CACHED["nc"] = build_program()
        _CACHED["consts"] = _dft_constants()
    return _CACHED["nc"], _CACHED["consts"]


def kernel(x: np.ndarray) -> np.ndarray:
    assert x.shape == (B_FULL, C2, H, W) and x.dtype == np.float32
    nc, (Dr, Di) = _get_program()
    x = np.ascontiguousarray(x)
    in_maps = [
        {"x": x[BPC * k : BPC * (k + 1)], "dr": Dr, "di": Di}
        for k in range(N_CORES)
    ]
    res = run_bass_kernel_spmd(nc, in_maps, list(range(N_CORES)))
    out = np.concatenate(
        [res.results[k]["y"] for k in range(N_CORES)], axis=0
    )
    return out.astype(np.float32, copy=False)


if __name__ == "__main__":
    rng = np.random.default_rng(0)
    x = rng.standard_normal((B_FULL, C2, H, W)).astype(np.float32)
    y = kernel(x)
    print("kernel output", y.shape, y.dtype)
